# revision 1
# baseline (speedup 1.0000x reference)
"""Causal single-head attention on 8 Trainium2 NeuronCores (Bass/Tile).

Problem: X [4, 2048, 1024] f32; W_q/W_k/W_v [1024, 1024] f32.
out[b] = softmax(mask((X[b] Wq)(X[b] Wk)^T / 32)) (X[b] Wv)

Sharding: 8 cores = 4 batches x 2 query-halves. Core c = 2b + h handles
batch b, query rows [1024h, 1024h + 1024). Each core gets X[b]^T (d-major,
for K/V projections of all 2048 keys), its own query column slice of X[b]^T,
the three weight matrices, and a per-core causal 0/1 "band" matrix from which
every (k-tile, q-chunk) mask is a compile-time slice. One uniform SPMD
program runs on all 8 cores; causality differences live entirely in data.

Layout strategy (all contractions keep the contracted dim on partitions):
  A: KT[e,s] = Wk^T X^T, QT[e,q] = Wq^T XQ^T (f32r matmuls, bf16 results),
     V[s,e] = X Wv.
  B: sT[k,q] = KT(:,ktile)^T-block @ QT  (scores, transposed: k on partitions)
     w = exp(sT/32) * band-slice         (multiplicative causal mask, bf16)
     denom[q] = ones-matmul over w;  out[q,e] = (w^T-as-lhsT @ V) * 1/denom
The transposed-score layout makes the attention weights directly usable as
matmul lhsT for AV - no on-chip transposes at all.
"""

import sys

if "/opt/trn_rl_repo" not in sys.path:
    sys.path.insert(0, "/opt/trn_rl_repo")

import numpy as np

B, S, D = 4, 2048, 1024
H = S // 2  # query rows per core
P = 128
BANDW = 3072
N_CORES = 8

_cache = {}


def _build_nc():
    from concourse import bacc
    import concourse.mybir as mybir
    import concourse.tile as tile

    fp32 = mybir.dt.float32
    fp32r = mybir.dt.float32r
    bf16 = mybir.dt.bfloat16
    Exp = mybir.ActivationFunctionType.Exp

    nc = bacc.Bacc("TRN2", target_bir_lowering=False)

    xt_d = nc.dram_tensor("xt", [D, S], fp32r, kind="ExternalInput")
    xtq_d = nc.dram_tensor("xtq", [D, H], fp32r, kind="ExternalInput")
    wq_d = nc.dram_tensor("wq", [D, D], fp32r, kind="ExternalInput")
    wk_d = nc.dram_tensor("wk", [D, D], fp32r, kind="ExternalInput")
    wv_d = nc.dram_tensor("wv", [D, D], fp32r, kind="ExternalInput")
    band_d = nc.dram_tensor("band", [P, BANDW], bf16, kind="ExternalInput")
    out_d = nc.dram_tensor("out", [H, D], fp32, kind="ExternalOutput")

    xt3 = xt_d.rearrange("(o p) s -> p o s", p=P)
    xq3 = xtq_d.rearrange("(o p) q -> p o q", p=P)

    with tile.TileContext(nc) as tc:
        with tc.tile_pool(name="persist", bufs=1) as persist:
            QT = persist.tile([P, 8, H], bf16, tag="qt")
            KT = persist.tile([P, 8, S], bf16, tag="kt")
            V = persist.tile([P, 16, D], bf16, tag="v")
            band = persist.tile([P, BANDW], bf16, tag="band")
            ones = persist.tile([P, 1], bf16, tag="ones")
            nc.sync.dma_start(band[:], band_d[:])
            nc.vector.memset(ones[:], 1.0)

            # ---- Phase A1: V[s, e] = X @ Wv (contract d on partitions) ----
            with (
                tc.tile_pool(name="wvp", bufs=1) as wvp,
                tc.tile_pool(name="xtk", bufs=3) as xtkp,
                tc.tile_pool(name="psA", bufs=4, space="PSUM") as psA,
            ):
                Wv = wvp.tile([P, 8, D], fp32r, tag="wv")
                nc.sync.dma_start(Wv[:], wv_d.rearrange("(o p) e -> p o e", p=P))
                for kt in range(16):
                    xc = xtkp.tile([P, 8, P], fp32r, tag="xtk")
                    nc.sync.dma_start(xc[:], xt3[:, :, kt * P : (kt + 1) * P])
                    for ec in range(2):
                        psum = psA.tile([P, 512], fp32, tag="psA")
                        for d in range(8):
                            nc.tensor.matmul(
                                psum[:],
                                xc[:, d],
                                Wv[:, d, ec * 512 : (ec + 1) * 512],
                                start=(d == 0),
                                stop=(d == 7),
                            )
                        nc.any.tensor_copy(
                            out=V[:, kt, ec * 512 : (ec + 1) * 512], in_=psum[:]
                        )

            # ---- Phase A2: KT[e, s] and QT[e, q] ----
            with (
                tc.tile_pool(name="wqk", bufs=1) as wqkp,
                tc.tile_pool(name="xts", bufs=2) as xtsp,
                tc.tile_pool(name="psB", bufs=4, space="PSUM") as psA2,
            ):
                Wk = wqkp.tile([P, 8, D], fp32r, tag="wk")
                Wq = wqkp.tile([P, 8, D], fp32r, tag="wq")
                nc.sync.dma_start(Wk[:], wk_d.rearrange("(o p) e -> p o e", p=P))
                nc.sync.dma_start(Wq[:], wq_d.rearrange("(o p) e -> p o e", p=P))
                for sc in range(4):
                    xs = xtsp.tile([P, 8, 512], fp32r, tag="xts")
                    nc.sync.dma_start(xs[:], xt3[:, :, sc * 512 : (sc + 1) * 512])
                    for e in range(8):
                        psum = psA2.tile([P, 512], fp32, tag="psB")
                        for d in range(8):
                            nc.tensor.matmul(
                                psum[:],
                                Wk[:, d, e * P : (e + 1) * P],
                                xs[:, d],
                                start=(d == 0),
                                stop=(d == 7),
                            )
                        nc.any.tensor_copy(
                            out=KT[:, e, sc * 512 : (sc + 1) * 512], in_=psum[:]
                        )
                for qsc in range(2):
                    xs = xtsp.tile([P, 8, 512], fp32r, tag="xts")
                    nc.sync.dma_start(xs[:], xq3[:, :, qsc * 512 : (qsc + 1) * 512])
                    for e in range(8):
                        psum = psA2.tile([P, 512], fp32, tag="psB")
                        for d in range(8):
                            nc.tensor.matmul(
                                psum[:],
                                Wq[:, d, e * P : (e + 1) * P],
                                xs[:, d],
                                start=(d == 0),
                                stop=(d == 7),
                            )
                        nc.any.tensor_copy(
                            out=QT[:, e, qsc * 512 : (qsc + 1) * 512], in_=psum[:]
                        )

            # ---- Phase B: attention ----
            with (
                tc.tile_pool(name="wtp", bufs=2) as wtp,
                tc.tile_pool(name="outp", bufs=2) as outp,
                tc.tile_pool(name="small", bufs=4) as smallp,
                tc.tile_pool(name="psS", bufs=2, space="PSUM") as psS,
                tc.tile_pool(name="psAV", bufs=4, space="PSUM") as psAV,
                tc.tile_pool(name="psD", bufs=2, space="PSUM") as psD,
            ):
                for qc in range(2):
                    wt = wtp.tile([P, 16, 512], bf16, tag="wt")
                    for kt in range(16):
                        psum_s = psS.tile([P, 512], fp32, tag="psS")
                        for e in range(8):
                            nc.tensor.matmul(
                                psum_s[:],
                                KT[:, e, kt * P : (kt + 1) * P],
                                QT[:, e, qc * 512 : (qc + 1) * 512],
                                start=(e == 0),
                                stop=(e == 7),
                            )
                        # w = exp(s/32) * causal01  (band slice = the mask)
                        nc.scalar.activation(wt[:, kt], psum_s[:], Exp, scale=1 / 32.0)
                        x0 = 2048 + 512 * qc - P * kt
                        nc.vector.tensor_mul(
                            wt[:, kt], wt[:, kt], band[:, x0 : x0 + 512]
                        )
                    for qt in range(4):
                        qs = slice(qt * P, (qt + 1) * P)
                        out_sb = outp.tile([P, D], fp32, tag="out")
                        psum_d = psD.tile([P, 1], fp32, tag="psD")
                        for kt in range(16):
                            nc.tensor.matmul(
                                psum_d[:],
                                wt[:, kt, qs],
                                ones[:],
                                start=(kt == 0),
                                stop=(kt == 15),
                            )
                        recip = smallp.tile([P, 1], fp32, tag="recip")
                        nc.vector.reciprocal(recip[:], psum_d[:])
                        for ec in range(2):
                            psum_av = psAV.tile([P, 512], fp32, tag="psAV")
                            for kt in range(16):
                                nc.tensor.matmul(
                                    psum_av[:],
                                    wt[:, kt, qs],
                                    V[:, kt, ec * 512 : (ec + 1) * 512],
                                    start=(kt == 0),
                                    stop=(kt == 15),
                                )
                            nc.vector.tensor_scalar_mul(
                                out_sb[:, ec * 512 : (ec + 1) * 512],
                                psum_av[:],
                                recip[:],
                            )
                        row = (qc * 4 + qt) * P
                        nc.sync.dma_start(out_d[row : row + P, :], out_sb[:])

    nc.compile()
    return nc


def _get_nc():
    if "nc" not in _cache:
        _cache["nc"] = _build_nc()
    return _cache["nc"]


def kernel(X, W_q, W_k, W_v, _run_kwargs=None, _results_out=None):
    import ml_dtypes
    from concourse.bass_utils import run_bass_kernel_spmd

    X = np.asarray(X, dtype=np.float32)
    W_q = np.asarray(W_q, dtype=np.float32)
    W_k = np.asarray(W_k, dtype=np.float32)
    W_v = np.asarray(W_v, dtype=np.float32)

    # per-core inputs
    xts = [np.ascontiguousarray(X[b].T) for b in range(B)]
    bands = []
    for h in range(2):
        x = np.arange(BANDW)[None, :]
        p = np.arange(P)[:, None]
        bands.append((x >= p + 2048 - H * h).astype(ml_dtypes.bfloat16))

    in_maps = []
    for c in range(N_CORES):
        b, h = divmod(c, 2)
        in_maps.append(
            {
                "xt": xts[b],
                "xtq": np.ascontiguousarray(xts[b][:, h * H : (h + 1) * H]),
                "wq": W_q,
                "wk": W_k,
                "wv": W_v,
                "band": bands[h],
            }
        )

    nc = _get_nc()
    res = run_bass_kernel_spmd(
        nc, in_maps, core_ids=list(range(N_CORES)), **(_run_kwargs or {})
    )
    if _results_out is not None:
        _results_out.append(res)

    out = np.empty((B, S, D), dtype=np.float32)
    for c in range(N_CORES):
        b, h = divmod(c, 2)
        out[b, h * H : (h + 1) * H, :] = res.results[c]["out"]
    return out


# revision 2
# speedup vs baseline: 1.3167x; 1.3167x over previous
"""Causal single-head attention on 8 Trainium2 NeuronCores (Bass/Tile).

Problem: X [4, 2048, 1024] f32; W_q/W_k/W_v [1024, 1024] f32.
out[b] = softmax(mask((X[b] Wq)(X[b] Wk)^T / 32)) (X[b] Wv)

Sharding: 8 cores = 4 batches x 2 query-stripe-sets. Core c = 2b + h handles
batch b and the 8 query tiles with global tile index {2t + h : t in 0..7}
(128-row tiles, interleaved). The stripe interleave makes the causal work
per local tile t (valid k-tiles = 2t + h + 1, rounded up to 2t + 2)
identical across cores, so one uniform SPMD program runs everywhere and all
causality differences live in data (a per-core 0/1 "band" matrix whose
compile-time slices are the per-(k-tile, q-tile) masks).

All matmul contractions keep the contracted dim on partitions:
  A: KT[e,s] = Wk^T X^T and QT[e,q] = Wq^T XQ^T (weights as lhsT),
     V[s,e] = X Wv (X^T chunks as lhsT). bf16 in/out, fp32 PSUM.
  B: sT[k,q] = KT-block^T @ QT (scores transposed: k on partitions)
     w = exp(sT/32) * band-slice  (multiplicative causal mask)
     denom[q] = ones-matmul over w; out[q,e] = (w-as-lhsT @ V) * 1/denom
The transposed-score layout makes the attention weights directly usable as
matmul lhsT for AV - no on-chip transposes at all.
"""

import sys

if "/opt/trn_rl_repo" not in sys.path:
    sys.path.insert(0, "/opt/trn_rl_repo")

import numpy as np

B, S, D = 4, 2048, 1024
H = S // 2  # query rows per core
P = 128
BANDW = 3968
N_CORES = 8

_cache = {}


def _build_nc():
    from concourse import bacc
    import concourse.mybir as mybir
    import concourse.tile as tile

    fp32 = mybir.dt.float32
    bf16 = mybir.dt.bfloat16
    Exp = mybir.ActivationFunctionType.Exp

    nc = bacc.Bacc("TRN2", target_bir_lowering=False)

    xt_d = nc.dram_tensor("xt", [D, S], bf16, kind="ExternalInput")
    xtq_d = nc.dram_tensor("xtq", [D, H], bf16, kind="ExternalInput")
    wq_d = nc.dram_tensor("wq", [D, D], bf16, kind="ExternalInput")
    wk_d = nc.dram_tensor("wk", [D, D], bf16, kind="ExternalInput")
    wv_d = nc.dram_tensor("wv", [D, D], bf16, kind="ExternalInput")
    band_d = nc.dram_tensor("band", [P, BANDW], bf16, kind="ExternalInput")
    out_d = nc.dram_tensor("out", [H, D], fp32, kind="ExternalOutput")

    xt3 = xt_d.rearrange("(o p) s -> p o s", p=P)
    xq3 = xtq_d.rearrange("(o p) q -> p o q", p=P)
    wq3 = wq_d.rearrange("(o p) e -> p o e", p=P)
    wk3 = wk_d.rearrange("(o p) e -> p o e", p=P)
    wv3 = wv_d.rearrange("(o p) e -> p o e", p=P)

    with tile.TileContext(nc) as tc:
        with tc.tile_pool(name="persist", bufs=1) as persist:
            QT = persist.tile([P, 8, H], bf16, tag="qt")
            KT = persist.tile([P, 8, S], bf16, tag="kt")
            V = persist.tile([P, 16, D], bf16, tag="v")
            band = persist.tile([P, BANDW], bf16, tag="band")
            ones = persist.tile([P, 1], bf16, tag="ones")
            nc.sync.dma_start(band[:], band_d[:])
            nc.vector.memset(ones[:], 1.0)

            # ---- Phase A: projections (all bf16 matmuls, fp32 PSUM) ----
            with (
                tc.tile_pool(name="wts", bufs=1) as wp,
                tc.tile_pool(name="xts", bufs=2) as xtsp,
                tc.tile_pool(name="psA", bufs=4, space="PSUM") as psA,
            ):
                Wk = wp.tile([P, 8, D], bf16, tag="wk")
                Wv = wp.tile([P, 8, D], bf16, tag="wv")
                Wq = wp.tile([P, 8, D], bf16, tag="wq")
                # striped weight DMAs so accumulation can begin per-d ASAP
                for d in range(8):
                    nc.sync.dma_start(Wk[:, d], wk3[:, d])
                for d in range(8):
                    nc.sync.dma_start(Wv[:, d], wv3[:, d])
                for d in range(8):
                    nc.sync.dma_start(Wq[:, d], wq3[:, d])

                for sc in range(4):
                    xs = xtsp.tile([P, 8, 512], bf16, tag="xts")
                    for d in range(8):
                        nc.sync.dma_start(
                            xs[:, d], xt3[:, d, sc * 512 : (sc + 1) * 512]
                        )
                    # KT[e, s-chunk] = Wk^T @ X^T chunk
                    for e in range(8):
                        psum = psA.tile([P, 512], fp32, tag="psA")
                        for d in range(8):
                            nc.tensor.matmul(
                                psum[:],
                                Wk[:, d, e * P : (e + 1) * P],
                                xs[:, d],
                                start=(d == 0),
                                stop=(d == 7),
                            )
                        nc.any.tensor_copy(
                            out=KT[:, e, sc * 512 : (sc + 1) * 512], in_=psum[:]
                        )
                    # V[k-tile, e] = X chunk @ Wv  (X^T slice as lhsT)
                    for kti in range(4):
                        kt = 4 * sc + kti
                        for ec in range(2):
                            psum = psA.tile([P, 512], fp32, tag="psA")
                            for d in range(8):
                                nc.tensor.matmul(
                                    psum[:],
                                    xs[:, d, kti * P : (kti + 1) * P],
                                    Wv[:, d, ec * 512 : (ec + 1) * 512],
                                    start=(d == 0),
                                    stop=(d == 7),
                                )
                            nc.any.tensor_copy(
                                out=V[:, kt, ec * 512 : (ec + 1) * 512], in_=psum[:]
                            )
                # QT[e, q-chunk] = Wq^T @ XQ^T chunk
                for qsc in range(2):
                    xs = xtsp.tile([P, 8, 512], bf16, tag="xts")
                    for d in range(8):
                        nc.sync.dma_start(
                            xs[:, d], xq3[:, d, qsc * 512 : (qsc + 1) * 512]
                        )
                    for e in range(8):
                        psum = psA.tile([P, 512], fp32, tag="psA")
                        for d in range(8):
                            nc.tensor.matmul(
                                psum[:],
                                Wq[:, d, e * P : (e + 1) * P],
                                xs[:, d],
                                start=(d == 0),
                                stop=(d == 7),
                            )
                        nc.any.tensor_copy(
                            out=QT[:, e, qsc * 512 : (qsc + 1) * 512], in_=psum[:]
                        )

            # ---- Phase B: attention (causal, stripe-balanced) ----
            with (
                tc.tile_pool(name="wtp", bufs=2) as wtp,
                tc.tile_pool(name="outp", bufs=2) as outp,
                tc.tile_pool(name="small", bufs=4) as smallp,
                tc.tile_pool(name="psS", bufs=2, space="PSUM") as psS,
                tc.tile_pool(name="psAV", bufs=4, space="PSUM") as psAV,
                tc.tile_pool(name="psD", bufs=2, space="PSUM") as psD,
            ):
                for qc in range(2):
                    # local q-tiles 4qc..4qc+3; global tile of local t = 2t+h.
                    # k-extent for the chunk: kt < 8qc+8 (max over its tiles).
                    nkt = 8 * qc + 8
                    wt = wtp.tile([P, 16, 512], bf16, tag="wt")
                    for kt in range(nkt):
                        psum_s = psS.tile([P, 512], fp32, tag="psS")
                        for e in range(8):
                            nc.tensor.matmul(
                                psum_s[:],
                                KT[:, e, kt * P : (kt + 1) * P],
                                QT[:, e, qc * 512 : (qc + 1) * 512],
                                start=(e == 0),
                                stop=(e == 7),
                            )
                        nc.scalar.activation(wt[:, kt], psum_s[:], Exp, scale=1 / 32.0)
                        # causal mask: only k-tiles {2t, 2t+1} of q-tile t=kt//2
                        # can straddle the diagonal; earlier k-tiles are fully
                        # valid, later ones are never read by AV.
                        t = kt // 2
                        if 4 * qc <= t < 4 * qc + 4:
                            ji = (t - 4 * qc) * P
                            x0 = 2048 - P * kt + 256 * t
                            nc.vector.tensor_mul(
                                wt[:, kt, ji : ji + P],
                                wt[:, kt, ji : ji + P],
                                band[:, x0 : x0 + P],
                            )
                    for ti in range(4):
                        t = 4 * qc + ti  # local q-tile; global tile 2t+h
                        nkt_t = 2 * t + 2  # causal k-tiles for this tile
                        qs = slice(ti * P, (ti + 1) * P)
                        out_sb = outp.tile([P, D], fp32, tag="out")
                        psum_d = psD.tile([P, 1], fp32, tag="psD")
                        for kt in range(nkt_t):
                            nc.tensor.matmul(
                                psum_d[:],
                                wt[:, kt, qs],
                                ones[:],
                                start=(kt == 0),
                                stop=(kt == nkt_t - 1),
                            )
                        recip = smallp.tile([P, 1], fp32, tag="recip")
                        nc.vector.reciprocal(recip[:], psum_d[:])
                        for ec in range(2):
                            psum_av = psAV.tile([P, 512], fp32, tag="psAV")
                            for kt in range(nkt_t):
                                nc.tensor.matmul(
                                    psum_av[:],
                                    wt[:, kt, qs],
                                    V[:, kt, ec * 512 : (ec + 1) * 512],
                                    start=(kt == 0),
                                    stop=(kt == nkt_t - 1),
                                )
                            nc.vector.tensor_scalar_mul(
                                out_sb[:, ec * 512 : (ec + 1) * 512],
                                psum_av[:],
                                recip[:],
                            )
                        row = t * P
                        nc.sync.dma_start(out_d[row : row + P, :], out_sb[:])

    nc.compile()
    return nc


def _get_nc():
    if "nc" not in _cache:
        _cache["nc"] = _build_nc()
    return _cache["nc"]


def _stripe_cols(h):
    return np.concatenate(
        [np.arange(P * (2 * t + h), P * (2 * t + h) + P) for t in range(8)]
    )


def kernel(X, W_q, W_k, W_v, _run_kwargs=None, _results_out=None):
    import ml_dtypes
    from concourse.bass_utils import run_bass_kernel_spmd

    bf = ml_dtypes.bfloat16
    X = np.asarray(X, dtype=np.float32)
    wq16 = np.asarray(W_q, dtype=np.float32).astype(bf)
    wk16 = np.asarray(W_k, dtype=np.float32).astype(bf)
    wv16 = np.asarray(W_v, dtype=np.float32).astype(bf)

    xts = [np.ascontiguousarray(X[b].T).astype(bf) for b in range(B)]
    bands = []
    cols = [_stripe_cols(0), _stripe_cols(1)]
    for h in range(2):
        x = np.arange(BANDW)[None, :]
        p = np.arange(P)[:, None]
        bands.append((x >= p + 2048 - P * h).astype(bf))

    in_maps = []
    for c in range(N_CORES):
        b, h = divmod(c, 2)
        in_maps.append(
            {
                "xt": xts[b],
                "xtq": np.ascontiguousarray(xts[b][:, cols[h]]),
                "wq": wq16,
                "wk": wk16,
                "wv": wv16,
                "band": bands[h],
            }
        )

    nc = _get_nc()
    res = run_bass_kernel_spmd(
        nc, in_maps, core_ids=list(range(N_CORES)), **(_run_kwargs or {})
    )
    if _results_out is not None:
        _results_out.append(res)

    out = np.empty((B, S, D), dtype=np.float32)
    for c in range(N_CORES):
        b, h = divmod(c, 2)
        out[b, cols[h], :] = res.results[c]["out"]
    return out


# revision 4
# speedup vs baseline: 1.4175x; 1.0765x over previous
"""Causal single-head attention on 8 Trainium2 NeuronCores (Bass/Tile).

Problem: X [4, 2048, 1024] f32; W_q/W_k/W_v [1024, 1024] f32.
out[b] = softmax(mask((X[b] Wq)(X[b] Wk)^T / 32)) (X[b] Wv)

Sharding: 8 cores = 4 batches x 2 query-stripe-sets. Core c = 2b + h handles
batch b and the 8 query tiles with global tile index {2t + h : t in 0..7}
(128-row tiles, interleaved). The stripe interleave makes the causal work
per local tile t (valid k-tiles = 2t + h + 1, rounded up to 2t + 2)
identical across cores, so one uniform SPMD program runs everywhere and all
causality differences live in data (a per-core 0/1 "band" matrix whose
compile-time slices are the per-(k-tile, q-tile) masks).

All matmul contractions keep the contracted dim on partitions:
  A: KT[e,s] = Wk^T X^T and QT[e,q] = Wq^T XQ^T (weights as lhsT),
     V[s,e] = X Wv (X^T chunks as lhsT). bf16 in/out, fp32 PSUM.
  B: sT[k,q] = KT-block^T @ QT (scores transposed: k on partitions)
     w = exp(sT/32) * band-slice  (multiplicative causal mask)
     denom[q] = ones-matmul over w; out[q,e] = (w-as-lhsT @ V) * 1/denom
The transposed-score layout makes the attention weights directly usable as
matmul lhsT for AV - no on-chip transposes at all.
"""

import sys

if "/opt/trn_rl_repo" not in sys.path:
    sys.path.insert(0, "/opt/trn_rl_repo")

import numpy as np

B, S, D = 4, 2048, 1024
H = S // 2  # query rows per core
P = 128
BANDW = 3968
N_CORES = 8

_cache = {}


def _build_nc():
    from concourse import bacc
    import concourse.mybir as mybir
    import concourse.tile as tile

    fp32 = mybir.dt.float32
    bf16 = mybir.dt.bfloat16
    Exp = mybir.ActivationFunctionType.Exp

    nc = bacc.Bacc("TRN2", target_bir_lowering=False)

    xt_d = nc.dram_tensor("xt", [D, S], bf16, kind="ExternalInput")
    xtq_d = nc.dram_tensor("xtq", [D, H], bf16, kind="ExternalInput")
    wq_d = nc.dram_tensor("wq", [D, D], bf16, kind="ExternalInput")
    wk_d = nc.dram_tensor("wk", [D, D], bf16, kind="ExternalInput")
    wv_d = nc.dram_tensor("wv", [D, D], bf16, kind="ExternalInput")
    band_d = nc.dram_tensor("band", [P, BANDW], bf16, kind="ExternalInput")
    out_d = nc.dram_tensor("out", [H, D], fp32, kind="ExternalOutput")

    xt3 = xt_d.rearrange("(o p) s -> p o s", p=P)
    xq3 = xtq_d.rearrange("(o p) q -> p o q", p=P)
    wq3 = wq_d.rearrange("(o p) e -> p o e", p=P)
    wk3 = wk_d.rearrange("(o p) e -> p o e", p=P)
    wv3 = wv_d.rearrange("(o p) e -> p o e", p=P)

    with tile.TileContext(nc) as tc:
        with tc.tile_pool(name="persist", bufs=1) as persist:
            QT = persist.tile([P, 8, H], bf16, tag="qt")
            KT = persist.tile([P, 8, S], bf16, tag="kt")
            V = persist.tile([P, 16, D], bf16, tag="v")
            band = persist.tile([P, BANDW], bf16, tag="band")
            ones = persist.tile([P, 1], bf16, tag="ones")
            nc.vector.memset(ones[:], 1.0)

            # ---- Phase A: projections (all bf16 matmuls, fp32 PSUM) ----
            with (
                tc.tile_pool(name="wts", bufs=1) as wp,
                tc.tile_pool(name="xts", bufs=2) as xtsp,
                tc.tile_pool(name="psA", bufs=4, space="PSUM") as psA,
            ):
                Wk = wp.tile([P, 8, D], bf16, tag="wk")
                Wv = wp.tile([P, 8, D], bf16, tag="wv")
                Wq = wp.tile([P, 8, D], bf16, tag="wq")

                for sc in range(4):
                    xs = xtsp.tile([P, 8, 512], bf16, tag="xts")
                    for d in range(8):
                        nc.sync.dma_start(
                            xs[:, d], xt3[:, d, sc * 512 : (sc + 1) * 512]
                        )
                    # DMA-issue-order pacing: first X chunk, then Wk (needed
                    # first), then Wv / Wq / band trickle in behind compute.
                    if sc == 0:
                        for d in range(8):
                            nc.sync.dma_start(Wk[:, d], wk3[:, d])
                        for d in range(8):
                            nc.sync.dma_start(Wv[:, d], wv3[:, d])
                    elif sc == 1:
                        for d in range(8):
                            nc.sync.dma_start(Wq[:, d], wq3[:, d])
                    elif sc == 2:
                        nc.sync.dma_start(band[:], band_d[:])
                    # KT[e, s-chunk] = Wk^T @ X^T chunk
                    for e in range(8):
                        psum = psA.tile([P, 512], fp32, tag="psA")
                        for d in range(8):
                            nc.tensor.matmul(
                                psum[:],
                                Wk[:, d, e * P : (e + 1) * P],
                                xs[:, d],
                                start=(d == 0),
                                stop=(d == 7),
                            )
                        nc.any.tensor_copy(
                            out=KT[:, e, sc * 512 : (sc + 1) * 512], in_=psum[:]
                        )
                    # V[k-tile, e] = X chunk @ Wv  (X^T slice as lhsT)
                    for kti in range(4):
                        kt = 4 * sc + kti
                        for ec in range(2):
                            psum = psA.tile([P, 512], fp32, tag="psA")
                            for d in range(8):
                                nc.tensor.matmul(
                                    psum[:],
                                    xs[:, d, kti * P : (kti + 1) * P],
                                    Wv[:, d, ec * 512 : (ec + 1) * 512],
                                    start=(d == 0),
                                    stop=(d == 7),
                                )
                            nc.any.tensor_copy(
                                out=V[:, kt, ec * 512 : (ec + 1) * 512], in_=psum[:]
                            )
                # QT[e, q-chunk] = Wq^T @ XQ^T chunk
                for qsc in range(2):
                    xs = xtsp.tile([P, 8, 512], bf16, tag="xts")
                    for d in range(8):
                        nc.sync.dma_start(
                            xs[:, d], xq3[:, d, qsc * 512 : (qsc + 1) * 512]
                        )
                    for e in range(8):
                        psum = psA.tile([P, 512], fp32, tag="psA")
                        for d in range(8):
                            nc.tensor.matmul(
                                psum[:],
                                Wq[:, d, e * P : (e + 1) * P],
                                xs[:, d],
                                start=(d == 0),
                                stop=(d == 7),
                            )
                        nc.any.tensor_copy(
                            out=QT[:, e, qsc * 512 : (qsc + 1) * 512], in_=psum[:]
                        )

            # ---- Phase B: attention (causal, stripe-balanced) ----
            with (
                tc.tile_pool(name="wtp", bufs=2) as wtp,
                tc.tile_pool(name="outp", bufs=2) as outp,
                tc.tile_pool(name="small", bufs=4) as smallp,
                tc.tile_pool(name="psS", bufs=2, space="PSUM") as psS,
                tc.tile_pool(name="psAV", bufs=4, space="PSUM") as psAV,
                tc.tile_pool(name="psD", bufs=2, space="PSUM") as psD,
            ):
                for qc in range(2):
                    # local q-tiles 4qc..4qc+3; global tile of local t = 2t+h.
                    # k-extent for the chunk: kt < 8qc+8 (max over its tiles).
                    nkt = 8 * qc + 8
                    wt = wtp.tile([P, 16, 512], bf16, tag="wt")
                    for kt in range(nkt):
                        psum_s = psS.tile([P, 512], fp32, tag="psS")
                        for e in range(8):
                            nc.tensor.matmul(
                                psum_s[:],
                                KT[:, e, kt * P : (kt + 1) * P],
                                QT[:, e, qc * 512 : (qc + 1) * 512],
                                start=(e == 0),
                                stop=(e == 7),
                            )
                        nc.scalar.activation(wt[:, kt], psum_s[:], Exp, scale=1 / 32.0)
                        # causal mask: only k-tiles {2t, 2t+1} of q-tile t=kt//2
                        # can straddle the diagonal; earlier k-tiles are fully
                        # valid, later ones are never read by AV.
                        t = kt // 2
                        if 4 * qc <= t < 4 * qc + 4:
                            ji = (t - 4 * qc) * P
                            x0 = 2048 - P * kt + 256 * t
                            nc.vector.tensor_mul(
                                wt[:, kt, ji : ji + P],
                                wt[:, kt, ji : ji + P],
                                band[:, x0 : x0 + P],
                            )
                    for ti in range(4):
                        t = 4 * qc + ti  # local q-tile; global tile 2t+h
                        nkt_t = 2 * t + 2  # causal k-tiles for this tile
                        qs = slice(ti * P, (ti + 1) * P)
                        out_sb = outp.tile([P, D], fp32, tag="out")
                        psum_d = psD.tile([P, 1], fp32, tag="psD")
                        for kt in range(nkt_t):
                            nc.tensor.matmul(
                                psum_d[:],
                                wt[:, kt, qs],
                                ones[:],
                                start=(kt == 0),
                                stop=(kt == nkt_t - 1),
                            )
                        recip = smallp.tile([P, 1], fp32, tag="recip")
                        nc.vector.reciprocal(recip[:], psum_d[:])
                        for ec in range(2):
                            psum_av = psAV.tile([P, 512], fp32, tag="psAV")
                            for kt in range(nkt_t):
                                nc.tensor.matmul(
                                    psum_av[:],
                                    wt[:, kt, qs],
                                    V[:, kt, ec * 512 : (ec + 1) * 512],
                                    start=(kt == 0),
                                    stop=(kt == nkt_t - 1),
                                )
                            nc.vector.tensor_scalar_mul(
                                out_sb[:, ec * 512 : (ec + 1) * 512],
                                psum_av[:],
                                recip[:],
                            )
                        row = t * P
                        nc.sync.dma_start(out_d[row : row + P, :], out_sb[:])

    nc.compile()
    return nc


def _get_nc():
    if "nc" not in _cache:
        _cache["nc"] = _build_nc()
    return _cache["nc"]


def _stripe_cols(h):
    return np.concatenate(
        [np.arange(P * (2 * t + h), P * (2 * t + h) + P) for t in range(8)]
    )


def kernel(X, W_q, W_k, W_v, _run_kwargs=None, _results_out=None):
    import ml_dtypes
    from concourse.bass_utils import run_bass_kernel_spmd

    bf = ml_dtypes.bfloat16
    X = np.asarray(X, dtype=np.float32)
    wq16 = np.asarray(W_q, dtype=np.float32).astype(bf)
    wk16 = np.asarray(W_k, dtype=np.float32).astype(bf)
    wv16 = np.asarray(W_v, dtype=np.float32).astype(bf)

    xts = [np.ascontiguousarray(X[b].T).astype(bf) for b in range(B)]
    bands = []
    cols = [_stripe_cols(0), _stripe_cols(1)]
    for h in range(2):
        x = np.arange(BANDW)[None, :]
        p = np.arange(P)[:, None]
        bands.append((x >= p + 2048 - P * h).astype(bf))

    in_maps = []
    for c in range(N_CORES):
        b, h = divmod(c, 2)
        in_maps.append(
            {
                "xt": xts[b],
                "xtq": np.ascontiguousarray(xts[b][:, cols[h]]),
                "wq": wq16,
                "wk": wk16,
                "wv": wv16,
                "band": bands[h],
            }
        )

    nc = _get_nc()
    res = run_bass_kernel_spmd(
        nc, in_maps, core_ids=list(range(N_CORES)), **(_run_kwargs or {})
    )
    if _results_out is not None:
        _results_out.append(res)

    out = np.empty((B, S, D), dtype=np.float32)
    for c in range(N_CORES):
        b, h = divmod(c, 2)
        out[b, cols[h], :] = res.results[c]["out"]
    return out


# revision 8
# speedup vs baseline: 1.4310x; 1.0095x over previous
"""Causal single-head attention on 8 Trainium2 NeuronCores (Bass/Tile).

Problem: X [4, 2048, 1024] f32; W_q/W_k/W_v [1024, 1024] f32.
out[b] = softmax(mask((X[b] Wq)(X[b] Wk)^T / 32)) (X[b] Wv)

Sharding: 8 cores = 4 batches x 2 query-stripe-sets. Core c = 2b + h handles
batch b and the 8 query tiles with global tile index {2t + h : t in 0..7}
(128-row tiles, interleaved). The stripe interleave makes the causal work
per local tile t (valid k-tiles = 2t + h + 1, rounded up to 2t + 2)
identical across cores, so one uniform SPMD program runs everywhere and all
causality differences live in data (a per-core 0/1 "band" matrix whose
compile-time slices are the per-(k-tile, q-tile) masks).

All matmul contractions keep the contracted dim on partitions:
  A: KT[e,s] = Wk^T X^T and QT[e,q] = Wq^T XQ^T (weights as lhsT),
     V[s,e] = X Wv (X^T chunks as lhsT). bf16 in/out, fp32 PSUM.
  B: sT[k,q] = KT-block^T @ QT (scores transposed: k on partitions)
     w = exp(sT/32) * band-slice  (multiplicative causal mask)
     denom[q] = ones-matmul over w; out[q,e] = (w-as-lhsT @ V) * 1/denom
The transposed-score layout makes the attention weights directly usable as
matmul lhsT for AV - no on-chip transposes at all.
"""

import sys

if "/opt/trn_rl_repo" not in sys.path:
    sys.path.insert(0, "/opt/trn_rl_repo")

import numpy as np

B, S, D = 4, 2048, 1024
H = S // 2  # query rows per core
P = 128
BANDW = 3968
N_CORES = 8

_cache = {}


def _build_nc():
    from concourse import bacc
    import concourse.mybir as mybir
    import concourse.tile as tile

    fp32 = mybir.dt.float32
    bf16 = mybir.dt.bfloat16
    Exp = mybir.ActivationFunctionType.Exp

    nc = bacc.Bacc("TRN2", target_bir_lowering=False)

    xt_d = nc.dram_tensor("xt", [D, S], bf16, kind="ExternalInput")
    xtq_d = nc.dram_tensor("xtq", [D, H], bf16, kind="ExternalInput")
    wq_d = nc.dram_tensor("wq", [D, D], bf16, kind="ExternalInput")
    wk_d = nc.dram_tensor("wk", [D, D], bf16, kind="ExternalInput")
    wv_d = nc.dram_tensor("wv", [D, D], bf16, kind="ExternalInput")
    band_d = nc.dram_tensor("band", [P, BANDW], bf16, kind="ExternalInput")
    out_d = nc.dram_tensor("out", [H, D], fp32, kind="ExternalOutput")

    xt3 = xt_d.rearrange("(o p) s -> p o s", p=P)
    xq3 = xtq_d.rearrange("(o p) q -> p o q", p=P)
    wq3 = wq_d.rearrange("(o p) e -> p o e", p=P)
    wk3 = wk_d.rearrange("(o p) e -> p o e", p=P)
    wv3 = wv_d.rearrange("(o p) e -> p o e", p=P)

    with tile.TileContext(nc) as tc:
        with tc.tile_pool(name="persist", bufs=1) as persist:
            QT = persist.tile([P, 8, H], bf16, tag="qt")
            KT = persist.tile([P, 8, S], bf16, tag="kt")
            V = persist.tile([P, 16, D], bf16, tag="v")
            band = persist.tile([P, BANDW], bf16, tag="band")
            ones = persist.tile([P, 1], bf16, tag="ones")
            nc.vector.memset(ones[:], 1.0)

            # ---- Phase A: projections (all bf16 matmuls, fp32 PSUM) ----
            with (
                tc.tile_pool(name="wts", bufs=1) as wp,
                tc.tile_pool(name="xts", bufs=2) as xtsp,
                tc.tile_pool(name="psA", bufs=8, space="PSUM") as psA,
            ):
                Wk = wp.tile([P, 8, D], bf16, tag="wk")
                Wv = wp.tile([P, 8, D], bf16, tag="wv")
                Wq = wp.tile([P, 8, D], bf16, tag="wq")

                for sc in range(4):
                    xs = xtsp.tile([P, 8, 512], bf16, tag="xts")
                    for d in range(8):
                        nc.sync.dma_start(
                            xs[:, d], xt3[:, d, sc * 512 : (sc + 1) * 512]
                        )
                    # DMA-issue-order pacing: first X chunk, then Wk (needed
                    # first), then Wv / Wq / band trickle in behind compute.
                    if sc == 0:
                        for d in range(8):
                            nc.sync.dma_start(Wk[:, d], wk3[:, d])
                        for d in range(8):
                            nc.sync.dma_start(Wv[:, d], wv3[:, d])
                    elif sc == 1:
                        for d in range(8):
                            nc.sync.dma_start(Wq[:, d], wq3[:, d])
                    elif sc == 2:
                        nc.sync.dma_start(band[:], band_d[:])
                    # KT[e, s-chunk] = Wk^T @ X^T chunk. For sc==0 run the
                    # accumulation d-outer across 8 live PSUM banks so the
                    # first matmuls pipeline with the arriving DMA stripes.
                    if sc == 0:
                        psums = [
                            psA.tile([P, 512], fp32, tag="psA", name=f"psA{e}")
                            for e in range(8)
                        ]
                        for d in range(8):
                            for e in range(8):
                                nc.tensor.matmul(
                                    psums[e][:],
                                    Wk[:, d, e * P : (e + 1) * P],
                                    xs[:, d],
                                    start=(d == 0),
                                    stop=(d == 7),
                                )
                        for e in range(8):
                            nc.any.tensor_copy(
                                out=KT[:, e, sc * 512 : (sc + 1) * 512],
                                in_=psums[e][:],
                            )
                    else:
                        for e in range(8):
                            psum = psA.tile([P, 512], fp32, tag="psA")
                            for d in range(8):
                                nc.tensor.matmul(
                                    psum[:],
                                    Wk[:, d, e * P : (e + 1) * P],
                                    xs[:, d],
                                    start=(d == 0),
                                    stop=(d == 7),
                                )
                            nc.any.tensor_copy(
                                out=KT[:, e, sc * 512 : (sc + 1) * 512], in_=psum[:]
                            )
                    # V[k-tile, e] = X chunk @ Wv  (X^T slice as lhsT)
                    for kti in range(4):
                        kt = 4 * sc + kti
                        for ec in range(2):
                            psum = psA.tile([P, 512], fp32, tag="psA")
                            for d in range(8):
                                nc.tensor.matmul(
                                    psum[:],
                                    xs[:, d, kti * P : (kti + 1) * P],
                                    Wv[:, d, ec * 512 : (ec + 1) * 512],
                                    start=(d == 0),
                                    stop=(d == 7),
                                )
                            nc.any.tensor_copy(
                                out=V[:, kt, ec * 512 : (ec + 1) * 512], in_=psum[:]
                            )
                # QT[e, q-chunk] = Wq^T @ XQ^T chunk
                for qsc in range(2):
                    xs = xtsp.tile([P, 8, 512], bf16, tag="xts")
                    for d in range(8):
                        nc.sync.dma_start(
                            xs[:, d], xq3[:, d, qsc * 512 : (qsc + 1) * 512]
                        )
                    for e in range(8):
                        psum = psA.tile([P, 512], fp32, tag="psA")
                        for d in range(8):
                            nc.tensor.matmul(
                                psum[:],
                                Wq[:, d, e * P : (e + 1) * P],
                                xs[:, d],
                                start=(d == 0),
                                stop=(d == 7),
                            )
                        nc.any.tensor_copy(
                            out=QT[:, e, qsc * 512 : (qsc + 1) * 512], in_=psum[:]
                        )

            # ---- Phase B: attention (causal, stripe-balanced) ----
            with (
                tc.tile_pool(name="wtp", bufs=2) as wtp,
                tc.tile_pool(name="outp", bufs=2) as outp,
                tc.tile_pool(name="small", bufs=4) as smallp,
                tc.tile_pool(name="psS", bufs=2, space="PSUM") as psS,
                tc.tile_pool(name="psAV", bufs=4, space="PSUM") as psAV,
                tc.tile_pool(name="psD", bufs=2, space="PSUM") as psD,
            ):
                for qc in range(4):
                    # 256-wide q-chunk: local q-tiles {2qc, 2qc+1}; global
                    # tile of local t = 2t+h. k-extent: kt < 4qc+4.
                    nkt = 4 * qc + 4
                    wt = wtp.tile([P, 16, 256], bf16, tag="wt")
                    for kt in range(nkt):
                        psum_s = psS.tile([P, 256], fp32, tag="psS")
                        for e in range(8):
                            nc.tensor.matmul(
                                psum_s[:],
                                KT[:, e, kt * P : (kt + 1) * P],
                                QT[:, e, qc * 256 : (qc + 1) * 256],
                                start=(e == 0),
                                stop=(e == 7),
                            )
                        nc.scalar.activation(wt[:, kt], psum_s[:], Exp, scale=1 / 32.0)
                        # causal mask: only k-tiles {2t, 2t+1} of q-tile t=kt//2
                        # can straddle the diagonal; earlier k-tiles are fully
                        # valid, later ones are never read by AV.
                        t = kt // 2
                        if 2 * qc <= t < 2 * qc + 2:
                            ji = (t - 2 * qc) * P
                            x0 = 2048 - P * kt + 256 * t
                            nc.vector.tensor_mul(
                                wt[:, kt, ji : ji + P],
                                wt[:, kt, ji : ji + P],
                                band[:, x0 : x0 + P],
                            )
                    for ti in range(2):
                        t = 2 * qc + ti  # local q-tile; global tile 2t+h
                        nkt_t = 2 * t + 2  # causal k-tiles for this tile
                        qs = slice(ti * P, (ti + 1) * P)
                        out_sb = outp.tile([P, D], fp32, tag="out")
                        psum_d = psD.tile([P, 1], fp32, tag="psD")
                        for kt in range(nkt_t):
                            nc.tensor.matmul(
                                psum_d[:],
                                wt[:, kt, qs],
                                ones[:],
                                start=(kt == 0),
                                stop=(kt == nkt_t - 1),
                            )
                        recip = smallp.tile([P, 1], fp32, tag="recip")
                        nc.vector.reciprocal(recip[:], psum_d[:])
                        for ec in range(2):
                            psum_av = psAV.tile([P, 512], fp32, tag="psAV")
                            for kt in range(nkt_t):
                                nc.tensor.matmul(
                                    psum_av[:],
                                    wt[:, kt, qs],
                                    V[:, kt, ec * 512 : (ec + 1) * 512],
                                    start=(kt == 0),
                                    stop=(kt == nkt_t - 1),
                                )
                            nc.vector.tensor_scalar_mul(
                                out_sb[:, ec * 512 : (ec + 1) * 512],
                                psum_av[:],
                                recip[:],
                            )
                        row = t * P
                        nc.sync.dma_start(out_d[row : row + P, :], out_sb[:])

    nc.compile()
    return nc


def _get_nc():
    if "nc" not in _cache:
        _cache["nc"] = _build_nc()
    return _cache["nc"]


def _stripe_cols(h):
    return np.concatenate(
        [np.arange(P * (2 * t + h), P * (2 * t + h) + P) for t in range(8)]
    )


def kernel(X, W_q, W_k, W_v, _run_kwargs=None, _results_out=None):
    import ml_dtypes
    from concourse.bass_utils import run_bass_kernel_spmd

    bf = ml_dtypes.bfloat16
    X = np.asarray(X, dtype=np.float32)
    wq16 = np.asarray(W_q, dtype=np.float32).astype(bf)
    wk16 = np.asarray(W_k, dtype=np.float32).astype(bf)
    wv16 = np.asarray(W_v, dtype=np.float32).astype(bf)

    xts = [np.ascontiguousarray(X[b].T).astype(bf) for b in range(B)]
    bands = []
    cols = [_stripe_cols(0), _stripe_cols(1)]
    for h in range(2):
        x = np.arange(BANDW)[None, :]
        p = np.arange(P)[:, None]
        bands.append((x >= p + 2048 - P * h).astype(bf))

    in_maps = []
    for c in range(N_CORES):
        b, h = divmod(c, 2)
        in_maps.append(
            {
                "xt": xts[b],
                "xtq": np.ascontiguousarray(xts[b][:, cols[h]]),
                "wq": wq16,
                "wk": wk16,
                "wv": wv16,
                "band": bands[h],
            }
        )

    nc = _get_nc()
    res = run_bass_kernel_spmd(
        nc, in_maps, core_ids=list(range(N_CORES)), **(_run_kwargs or {})
    )
    if _results_out is not None:
        _results_out.append(res)

    out = np.empty((B, S, D), dtype=np.float32)
    for c in range(N_CORES):
        b, h = divmod(c, 2)
        out[b, cols[h], :] = res.results[c]["out"]
    return out


# revision 9
# speedup vs baseline: 1.5814x; 1.1051x over previous
"""Causal single-head attention on 8 Trainium2 NeuronCores (Bass/Tile).

Problem: X [4, 2048, 1024] f32; W_q/W_k/W_v [1024, 1024] f32.
out[b] = softmax(mask((X[b] Wq)(X[b] Wk)^T / 32)) (X[b] Wv)

Sharding: 8 cores = 4 batches x 2 key-parity halves (partial softmax).
Core c = 2b + h owns batch b's key tiles {2j + h : j = 0..7} (128-row
tiles, interleaved so causal work per local tile j is j-independent across
cores). Each core projects K/V only for its own key tiles (K/V computed
once globally; only Q is duplicated - the cheapest possible duplication),
computes unnormalized partial attention over its keys, and returns the
partial numerator [2048, 1024] plus partial softmax denominators. The host
adds each pair's partials and divides. Since exp needs no max-subtraction
here (|scores/32| < ~4), partial softmax combines exactly.

One uniform SPMD program: per-core differences live in data only (which
key columns of X^T arrive in `xk`, and a [128, 256] 0/1 band mask whose
content encodes the core's parity for the diagonal score tiles).

All matmul contractions keep the contracted dim on partitions:
  A: KT[e,k] = Wk^T Xk^T, QT[e,q] = Wq^T X^T (weights as lhsT),
     V[k,e] = Xk Wv (Xk^T chunks as lhsT). bf16 in/out, fp32 PSUM.
  B: sT[k,q] = KT-tile^T @ QT (scores transposed: own keys on partitions)
     w = exp(sT/32) * band   (band only on the two diagonal q-tiles)
     den[q] = ones-matmul over w; num[q,e] = w-as-lhsT @ V
The transposed-score layout makes the attention weights directly usable as
matmul lhsT for the numerator - no on-chip transposes at all.
"""

import sys

if "/opt/trn_rl_repo" not in sys.path:
    sys.path.insert(0, "/opt/trn_rl_repo")

import numpy as np

B, S, D = 4, 2048, 1024
HK = S // 2  # own key rows per core
P = 128
N_CORES = 8
# column offset of attention-weight block j inside the packed wt tile
WOFF = [0] * 9
for _j in range(8):
    WOFF[_j + 1] = WOFF[_j] + (16 - 2 * _j) * P
WTW = WOFF[8]  # 9216

_cache = {}


def _build_nc():
    from concourse import bacc
    import concourse.mybir as mybir
    import concourse.tile as tile

    fp32 = mybir.dt.float32
    bf16 = mybir.dt.bfloat16
    Exp = mybir.ActivationFunctionType.Exp

    nc = bacc.Bacc("TRN2", target_bir_lowering=False)

    xk_d = nc.dram_tensor("xk", [D, HK], bf16, kind="ExternalInput")
    xq_d = nc.dram_tensor("xq", [D, S], bf16, kind="ExternalInput")
    wq_d = nc.dram_tensor("wq", [D, D], bf16, kind="ExternalInput")
    wk_d = nc.dram_tensor("wk", [D, D], bf16, kind="ExternalInput")
    wv_d = nc.dram_tensor("wv", [D, D], bf16, kind="ExternalInput")
    band_d = nc.dram_tensor("band", [P, 256], bf16, kind="ExternalInput")
    num_d = nc.dram_tensor("num", [S, D], fp32, kind="ExternalOutput")
    den_d = nc.dram_tensor("den", [P, 16], fp32, kind="ExternalOutput")

    xk3 = xk_d.rearrange("(o p) s -> p o s", p=P)
    xq3 = xq_d.rearrange("(o p) q -> p o q", p=P)
    wq3 = wq_d.rearrange("(o p) e -> p o e", p=P)
    wk3 = wk_d.rearrange("(o p) e -> p o e", p=P)
    wv3 = wv_d.rearrange("(o p) e -> p o e", p=P)

    with tile.TileContext(nc) as tc:
        with tc.tile_pool(name="persist", bufs=1) as persist:
            QT = persist.tile([P, 8, S], bf16, tag="qt")
            KT = persist.tile([P, 8, HK], bf16, tag="kt")
            V = persist.tile([P, 8, D], bf16, tag="v")
            band = persist.tile([P, 256], bf16, tag="band")
            ones = persist.tile([P, 1], bf16, tag="ones")
            nc.vector.memset(ones[:], 1.0)

            # ---- Phase A: projections (all bf16 matmuls, fp32 PSUM) ----
            with (
                tc.tile_pool(name="wts", bufs=1) as wp,
                tc.tile_pool(name="xts", bufs=2) as xtsp,
                tc.tile_pool(name="psA", bufs=8, space="PSUM") as psA,
            ):
                Wk = wp.tile([P, 8, D], bf16, tag="wk")
                Wv = wp.tile([P, 8, D], bf16, tag="wv")
                Wq = wp.tile([P, 8, D], bf16, tag="wq")

                for sc in range(2):
                    xs = xtsp.tile([P, 8, 512], bf16, tag="xts")
                    for d in range(8):
                        nc.sync.dma_start(
                            xs[:, d], xk3[:, d, sc * 512 : (sc + 1) * 512]
                        )
                    # DMA-issue-order pacing: X chunk first, then the weights
                    # needed soonest; the rest trickle in behind compute.
                    if sc == 0:
                        for d in range(8):
                            nc.sync.dma_start(Wk[:, d], wk3[:, d])
                        for d in range(8):
                            nc.sync.dma_start(Wv[:, d], wv3[:, d])
                    elif sc == 1:
                        for d in range(8):
                            nc.sync.dma_start(Wq[:, d], wq3[:, d])
                        nc.sync.dma_start(band[:], band_d[:])
                    # KT[e, k-chunk] = Wk^T @ Xk^T chunk. For sc==0 run the
                    # accumulation d-outer across 8 live PSUM banks so the
                    # first matmuls pipeline with the arriving DMA stripes.
                    if sc == 0:
                        psums = [
                            psA.tile([P, 512], fp32, tag="psA", name=f"psA{e}")
                            for e in range(8)
                        ]
                        for d in range(8):
                            for e in range(8):
                                nc.tensor.matmul(
                                    psums[e][:],
                                    Wk[:, d, e * P : (e + 1) * P],
                                    xs[:, d],
                                    start=(d == 0),
                                    stop=(d == 7),
                                )
                        for e in range(8):
                            nc.any.tensor_copy(
                                out=KT[:, e, sc * 512 : (sc + 1) * 512],
                                in_=psums[e][:],
                            )
                    else:
                        for e in range(8):
                            psum = psA.tile([P, 512], fp32, tag="psA")
                            for d in range(8):
                                nc.tensor.matmul(
                                    psum[:],
                                    Wk[:, d, e * P : (e + 1) * P],
                                    xs[:, d],
                                    start=(d == 0),
                                    stop=(d == 7),
                                )
                            nc.any.tensor_copy(
                                out=KT[:, e, sc * 512 : (sc + 1) * 512], in_=psum[:]
                            )
                    # V[k-tile, e] = Xk chunk @ Wv  (Xk^T slice as lhsT)
                    for kti in range(4):
                        kt = 4 * sc + kti
                        for ec in range(2):
                            psum = psA.tile([P, 512], fp32, tag="psA")
                            for d in range(8):
                                nc.tensor.matmul(
                                    psum[:],
                                    xs[:, d, kti * P : (kti + 1) * P],
                                    Wv[:, d, ec * 512 : (ec + 1) * 512],
                                    start=(d == 0),
                                    stop=(d == 7),
                                )
                            nc.any.tensor_copy(
                                out=V[:, kt, ec * 512 : (ec + 1) * 512], in_=psum[:]
                            )
                # QT[e, q-chunk] = Wq^T @ X^T chunk (all 2048 query rows)
                for qsc in range(4):
                    xs = xtsp.tile([P, 8, 512], bf16, tag="xts")
                    for d in range(8):
                        nc.sync.dma_start(
                            xs[:, d], xq3[:, d, qsc * 512 : (qsc + 1) * 512]
                        )
                    for e in range(8):
                        psum = psA.tile([P, 512], fp32, tag="psA")
                        for d in range(8):
                            nc.tensor.matmul(
                                psum[:],
                                Wq[:, d, e * P : (e + 1) * P],
                                xs[:, d],
                                start=(d == 0),
                                stop=(d == 7),
                            )
                        nc.any.tensor_copy(
                            out=QT[:, e, qsc * 512 : (qsc + 1) * 512], in_=psum[:]
                        )

            # ---- Phase B: partial attention over own key tiles ----
            with (
                tc.tile_pool(name="wtp", bufs=1) as wtp,
                tc.tile_pool(name="outp", bufs=2) as outp,
                tc.tile_pool(name="small", bufs=2) as smallp,
                tc.tile_pool(name="psS", bufs=2, space="PSUM") as psS,
                tc.tile_pool(name="psAV", bufs=4, space="PSUM") as psAV,
                tc.tile_pool(name="psD", bufs=2, space="PSUM") as psD,
            ):
                # scores + exp for all own key tiles j; q-range [256j, 2048)
                wt = wtp.tile([P, WTW], bf16, tag="wt")
                den_sb = smallp.tile([P, 16], fp32, tag="den")
                for j in range(8):
                    for ch in range(8 - j):
                        q0 = 256 * j + 256 * ch
                        psum_s = psS.tile([P, 256], fp32, tag="psS")
                        for e in range(8):
                            nc.tensor.matmul(
                                psum_s[:],
                                KT[:, e, j * P : (j + 1) * P],
                                QT[:, e, q0 : q0 + 256],
                                start=(e == 0),
                                stop=(e == 7),
                            )
                        wcol = WOFF[j] + 256 * ch
                        nc.scalar.activation(
                            wt[:, wcol : wcol + 256], psum_s[:], Exp, scale=1 / 32.0
                        )
                        if ch == 0:
                            # diagonal block: causal 0/1 mask (parity in data)
                            nc.vector.tensor_mul(
                                wt[:, wcol : wcol + 256],
                                wt[:, wcol : wcol + 256],
                                band[:],
                            )
                # partial numerator + denominator per global q-tile g
                for g in range(16):
                    nj = g // 2 + 1  # own key tiles j with 2j <= g
                    out_sb = outp.tile([P, D], fp32, tag="out")
                    psum_dn = psD.tile([P, 1], fp32, tag="psD")
                    for j in range(nj):
                        nc.tensor.matmul(
                            psum_dn[:],
                            wt[:, WOFF[j] + (g - 2 * j) * P :][:, :P],
                            ones[:],
                            start=(j == 0),
                            stop=(j == nj - 1),
                        )
                    nc.vector.tensor_copy(den_sb[:, g : g + 1], psum_dn[:])
                    for ec in range(2):
                        psum_av = psAV.tile([P, 512], fp32, tag="psAV")
                        for j in range(nj):
                            nc.tensor.matmul(
                                psum_av[:],
                                wt[:, WOFF[j] + (g - 2 * j) * P :][:, :P],
                                V[:, j, ec * 512 : (ec + 1) * 512],
                                start=(j == 0),
                                stop=(j == nj - 1),
                            )
                        nc.vector.tensor_copy(
                            out_sb[:, ec * 512 : (ec + 1) * 512], psum_av[:]
                        )
                    nc.sync.dma_start(num_d[g * P : (g + 1) * P, :], out_sb[:])
                nc.sync.dma_start(den_d[:], den_sb[:])

    nc.compile()
    return nc


def _get_nc():
    if "nc" not in _cache:
        _cache["nc"] = _build_nc()
    return _cache["nc"]


def _parity_cols(h):
    return np.concatenate(
        [np.arange(P * (2 * j + h), P * (2 * j + h) + P) for j in range(8)]
    )


def kernel(X, W_q, W_k, W_v, _run_kwargs=None, _results_out=None):
    import ml_dtypes
    from concourse.bass_utils import run_bass_kernel_spmd

    bf = ml_dtypes.bfloat16
    X = np.asarray(X, dtype=np.float32)
    wq16 = np.asarray(W_q, dtype=np.float32).astype(bf)
    wk16 = np.asarray(W_k, dtype=np.float32).astype(bf)
    wv16 = np.asarray(W_v, dtype=np.float32).astype(bf)

    xqs = [np.ascontiguousarray(X[b].T).astype(bf) for b in range(B)]
    cols = [_parity_cols(0), _parity_cols(1)]
    bands = []
    for h in range(2):
        x = np.arange(256)[None, :]
        p = np.arange(P)[:, None]
        bands.append((x >= p + P * h).astype(bf))

    in_maps = []
    for c in range(N_CORES):
        b, h = divmod(c, 2)
        in_maps.append(
            {
                "xk": np.ascontiguousarray(xqs[b][:, cols[h]]),
                "xq": xqs[b],
                "wq": wq16,
                "wk": wk16,
                "wv": wv16,
                "band": bands[h],
            }
        )

    nc = _get_nc()
    res = run_bass_kernel_spmd(
        nc, in_maps, core_ids=list(range(N_CORES)), **(_run_kwargs or {})
    )
    if _results_out is not None:
        _results_out.append(res)

    out = np.empty((B, S, D), dtype=np.float32)
    for b in range(B):
        re, ro = res.results[2 * b], res.results[2 * b + 1]
        num = re["num"] + ro["num"]
        den = (re["den"] + ro["den"]).T.reshape(S)  # row 128g+p <- den[p, g]
        out[b] = num / den[:, None]
    return out


# revision 11
# speedup vs baseline: 1.6070x; 1.0162x over previous
"""Causal single-head attention on 8 Trainium2 NeuronCores (Bass/Tile).

Problem: X [4, 2048, 1024] f32; W_q/W_k/W_v [1024, 1024] f32.
out[b] = softmax(mask((X[b] Wq)(X[b] Wk)^T / 32)) (X[b] Wv)

Sharding: 8 cores = 4 batches x 2 key-parity halves (partial softmax).
Core c = 2b + h owns batch b's key tiles {2j + h : j = 0..7} (128-row
tiles, interleaved so causal work per local tile j is j-independent across
cores). Each core projects K/V only for its own key tiles (K/V computed
once globally; only Q is duplicated - the cheapest possible duplication),
computes unnormalized partial attention over its keys, and returns the
partial numerator [2048, 1024] plus partial softmax denominators. The host
adds each pair's partials and divides. Since exp needs no max-subtraction
here (|scores/32| < ~4), partial softmax combines exactly.

One uniform SPMD program: per-core differences live in data only (which
key columns of X^T arrive in `xk`, and a [128, 256] 0/1 band mask whose
content encodes the core's parity for the diagonal score tiles).

All matmul contractions keep the contracted dim on partitions:
  A: KT[e,k] = Wk^T Xk^T, QT[e,q] = Wq^T X^T (weights as lhsT),
     V[k,e] = Xk Wv (Xk^T chunks as lhsT). bf16 in/out, fp32 PSUM.
  B: sT[k,q] = KT-tile^T @ QT (scores transposed: own keys on partitions)
     w = exp(sT/32) * band   (band only on the two diagonal q-tiles)
     den[q] = ones-matmul over w; num[q,e] = w-as-lhsT @ V
The transposed-score layout makes the attention weights directly usable as
matmul lhsT for the numerator - no on-chip transposes at all.
"""

import sys

if "/opt/trn_rl_repo" not in sys.path:
    sys.path.insert(0, "/opt/trn_rl_repo")

import numpy as np

B, S, D = 4, 2048, 1024
HK = S // 2  # own key rows per core
P = 128
N_CORES = 8
# column offset of attention-weight block j inside the packed wt tile
WOFF = [0] * 9
for _j in range(8):
    WOFF[_j + 1] = WOFF[_j] + (16 - 2 * _j) * P
WTW = WOFF[8]  # 9216

_cache = {}


def _build_nc():
    from concourse import bacc
    import concourse.mybir as mybir
    import concourse.tile as tile

    fp32 = mybir.dt.float32
    bf16 = mybir.dt.bfloat16
    Exp = mybir.ActivationFunctionType.Exp

    nc = bacc.Bacc("TRN2", target_bir_lowering=False)

    xk_d = nc.dram_tensor("xk", [D, HK], bf16, kind="ExternalInput")
    xq_d = nc.dram_tensor("xq", [D, S], bf16, kind="ExternalInput")
    wq_d = nc.dram_tensor("wq", [D, D], bf16, kind="ExternalInput")
    wk_d = nc.dram_tensor("wk", [D, D], bf16, kind="ExternalInput")
    wv_d = nc.dram_tensor("wv", [D, D], bf16, kind="ExternalInput")
    band_d = nc.dram_tensor("band", [P, 256], bf16, kind="ExternalInput")
    num_d = nc.dram_tensor("num", [S, D], fp32, kind="ExternalOutput")
    den_d = nc.dram_tensor("den", [P, 16], fp32, kind="ExternalOutput")

    xk3 = xk_d.rearrange("(o p) s -> p o s", p=P)
    xq3 = xq_d.rearrange("(o p) q -> p o q", p=P)
    wq3 = wq_d.rearrange("(o p) e -> p o e", p=P)
    wk3 = wk_d.rearrange("(o p) e -> p o e", p=P)
    wv3 = wv_d.rearrange("(o p) e -> p o e", p=P)

    with tile.TileContext(nc) as tc:
        with tc.tile_pool(name="persist", bufs=1) as persist:
            QT = persist.tile([P, 8, S], bf16, tag="qt")
            KT = persist.tile([P, 8, HK], bf16, tag="kt")
            V = persist.tile([P, 8, D], bf16, tag="v")
            band = persist.tile([P, 256], bf16, tag="band")
            ones = persist.tile([P, 1], bf16, tag="ones")
            nc.vector.memset(ones[:], 1.0)

            # ---- Phase A: projections (all bf16 matmuls, fp32 PSUM) ----
            with (
                tc.tile_pool(name="wts", bufs=1) as wp,
                tc.tile_pool(name="xts", bufs=2) as xtsp,
                tc.tile_pool(name="psA", bufs=8, space="PSUM") as psA,
            ):
                Wk = wp.tile([P, 8, D], bf16, tag="wk")
                Wv = wp.tile([P, 8, D], bf16, tag="wv")
                Wq = wp.tile([P, 8, D], bf16, tag="wq")

                for sc in range(2):
                    xs = xtsp.tile([P, 8, 512], bf16, tag="xts")
                    nc.sync.dma_start(xs[:], xk3[:, :, sc * 512 : (sc + 1) * 512])
                    # DMA-issue-order pacing: X chunk first, then the weights
                    # needed soonest; the rest trickle in behind compute.
                    # Batched DMAs: each dma_start costs ~650ns of serial
                    # queue-issue, so fewer+bigger wins.
                    if sc == 0:
                        nc.sync.dma_start(Wk[:], wk3[:])
                        nc.sync.dma_start(Wv[:], wv3[:])
                    elif sc == 1:
                        nc.sync.dma_start(Wq[:], wq3[:])
                        nc.sync.dma_start(band[:], band_d[:])
                    # KT[e, k-chunk] = Wk^T @ Xk^T chunk
                    for e in range(8):
                        psum = psA.tile([P, 512], fp32, tag="psA")
                        for d in range(8):
                            nc.tensor.matmul(
                                psum[:],
                                Wk[:, d, e * P : (e + 1) * P],
                                xs[:, d],
                                start=(d == 0),
                                stop=(d == 7),
                            )
                        nc.any.tensor_copy(
                            out=KT[:, e, sc * 512 : (sc + 1) * 512], in_=psum[:]
                        )
                    # V[k-tile, e] = Xk chunk @ Wv  (Xk^T slice as lhsT)
                    for kti in range(4):
                        kt = 4 * sc + kti
                        for ec in range(2):
                            psum = psA.tile([P, 512], fp32, tag="psA")
                            for d in range(8):
                                nc.tensor.matmul(
                                    psum[:],
                                    xs[:, d, kti * P : (kti + 1) * P],
                                    Wv[:, d, ec * 512 : (ec + 1) * 512],
                                    start=(d == 0),
                                    stop=(d == 7),
                                )
                            nc.any.tensor_copy(
                                out=V[:, kt, ec * 512 : (ec + 1) * 512], in_=psum[:]
                            )
                # QT[e, q-chunk] = Wq^T @ X^T chunk (all 2048 query rows)
                for qsc in range(4):
                    xs = xtsp.tile([P, 8, 512], bf16, tag="xts")
                    nc.sync.dma_start(xs[:], xq3[:, :, qsc * 512 : (qsc + 1) * 512])
                    for e in range(8):
                        psum = psA.tile([P, 512], fp32, tag="psA")
                        for d in range(8):
                            nc.tensor.matmul(
                                psum[:],
                                Wq[:, d, e * P : (e + 1) * P],
                                xs[:, d],
                                start=(d == 0),
                                stop=(d == 7),
                            )
                        nc.any.tensor_copy(
                            out=QT[:, e, qsc * 512 : (qsc + 1) * 512], in_=psum[:]
                        )

            # ---- Phase B: partial attention over own key tiles ----
            with (
                tc.tile_pool(name="wtp", bufs=1) as wtp,
                tc.tile_pool(name="outp", bufs=2) as outp,
                tc.tile_pool(name="small", bufs=2) as smallp,
                tc.tile_pool(name="psS", bufs=2, space="PSUM") as psS,
                tc.tile_pool(name="psAV", bufs=4, space="PSUM") as psAV,
                tc.tile_pool(name="psD", bufs=2, space="PSUM") as psD,
            ):
                # Interleaved: after key tile j's scores are exp'd, emit the
                # numerator/denominator for global q-tiles g = 2j and 2j+1
                # (they need only key tiles <= j). Keeps PE dense and spreads
                # the PSUM->SBUF copies across the whole phase.
                wt = wtp.tile([P, WTW], bf16, tag="wt")
                den_sb = smallp.tile([P, 16], fp32, tag="den")
                for j in range(8):
                    # scores + exp for own key tile j; q-range [256j, 2048)
                    for ch in range(8 - j):
                        q0 = 256 * j + 256 * ch
                        psum_s = psS.tile([P, 256], fp32, tag="psS")
                        for e in range(8):
                            nc.tensor.matmul(
                                psum_s[:],
                                KT[:, e, j * P : (j + 1) * P],
                                QT[:, e, q0 : q0 + 256],
                                start=(e == 0),
                                stop=(e == 7),
                            )
                        wcol = WOFF[j] + 256 * ch
                        nc.scalar.activation(
                            wt[:, wcol : wcol + 256], psum_s[:], Exp, scale=1 / 32.0
                        )
                        if ch == 0:
                            # diagonal block: causal 0/1 mask (parity in data)
                            nc.vector.tensor_mul(
                                wt[:, wcol : wcol + 256],
                                wt[:, wcol : wcol + 256],
                                band[:],
                            )
                    for g in (2 * j, 2 * j + 1):
                        nj = g // 2 + 1  # own key tiles jj with 2jj <= g
                        out_sb = outp.tile([P, D], fp32, tag="out")
                        psum_dn = psD.tile([P, 1], fp32, tag="psD")
                        for jj in range(nj):
                            nc.tensor.matmul(
                                psum_dn[:],
                                wt[:, WOFF[jj] + (g - 2 * jj) * P :][:, :P],
                                ones[:],
                                start=(jj == 0),
                                stop=(jj == nj - 1),
                            )
                        nc.any.tensor_copy(out=den_sb[:, g : g + 1], in_=psum_dn[:])
                        for ec in range(2):
                            psum_av = psAV.tile([P, 512], fp32, tag="psAV")
                            for jj in range(nj):
                                nc.tensor.matmul(
                                    psum_av[:],
                                    wt[:, WOFF[jj] + (g - 2 * jj) * P :][:, :P],
                                    V[:, jj, ec * 512 : (ec + 1) * 512],
                                    start=(jj == 0),
                                    stop=(jj == nj - 1),
                                )
                            nc.any.tensor_copy(
                                out=out_sb[:, ec * 512 : (ec + 1) * 512],
                                in_=psum_av[:],
                            )
                        nc.sync.dma_start(num_d[g * P : (g + 1) * P, :], out_sb[:])
                nc.sync.dma_start(den_d[:], den_sb[:])

    nc.compile()
    return nc


def _get_nc():
    if "nc" not in _cache:
        _cache["nc"] = _build_nc()
    return _cache["nc"]


def _parity_cols(h):
    return np.concatenate(
        [np.arange(P * (2 * j + h), P * (2 * j + h) + P) for j in range(8)]
    )


def kernel(X, W_q, W_k, W_v, _run_kwargs=None, _results_out=None):
    import ml_dtypes
    from concourse.bass_utils import run_bass_kernel_spmd

    bf = ml_dtypes.bfloat16
    X = np.asarray(X, dtype=np.float32)
    wq16 = np.asarray(W_q, dtype=np.float32).astype(bf)
    wk16 = np.asarray(W_k, dtype=np.float32).astype(bf)
    wv16 = np.asarray(W_v, dtype=np.float32).astype(bf)

    xqs = [np.ascontiguousarray(X[b].T).astype(bf) for b in range(B)]
    cols = [_parity_cols(0), _parity_cols(1)]
    bands = []
    for h in range(2):
        x = np.arange(256)[None, :]
        p = np.arange(P)[:, None]
        bands.append((x >= p + P * h).astype(bf))

    in_maps = []
    for c in range(N_CORES):
        b, h = divmod(c, 2)
        in_maps.append(
            {
                "xk": np.ascontiguousarray(xqs[b][:, cols[h]]),
                "xq": xqs[b],
                "wq": wq16,
                "wk": wk16,
                "wv": wv16,
                "band": bands[h],
            }
        )

    nc = _get_nc()
    res = run_bass_kernel_spmd(
        nc, in_maps, core_ids=list(range(N_CORES)), **(_run_kwargs or {})
    )
    if _results_out is not None:
        _results_out.append(res)

    out = np.empty((B, S, D), dtype=np.float32)
    for b in range(B):
        re, ro = res.results[2 * b], res.results[2 * b + 1]
        num = re["num"] + ro["num"]
        den = (re["den"] + ro["den"]).T.reshape(S)  # row 128g+p <- den[p, g]
        out[b] = num / den[:, None]
    return out


# revision 12
# speedup vs baseline: 1.6308x; 1.0148x over previous
"""Causal single-head attention on 8 Trainium2 NeuronCores (Bass/Tile).

Problem: X [4, 2048, 1024] f32; W_q/W_k/W_v [1024, 1024] f32.
out[b] = softmax(mask((X[b] Wq)(X[b] Wk)^T / 32)) (X[b] Wv)

Sharding: 8 cores = 4 batches x 2 key-parity halves (partial softmax).
Core c = 2b + h owns batch b's key tiles {2j + h : j = 0..7} (128-row
tiles, interleaved so causal work per local tile j is j-independent across
cores). Each core projects K/V only for its own key tiles (K/V computed
once globally; only Q is duplicated - the cheapest possible duplication),
computes unnormalized partial attention over its keys, and returns the
partial numerator [2048, 1024] plus partial softmax denominators. The host
adds each pair's partials and divides. Since exp needs no max-subtraction
here (|scores/32| < ~4), partial softmax combines exactly.

One uniform SPMD program: per-core differences live in data only (which
key columns of X^T arrive in `xk`, and a [128, 256] 0/1 band mask whose
content encodes the core's parity for the diagonal score tiles).

All matmul contractions keep the contracted dim on partitions:
  A: KT[e,k] = Wk^T Xk^T, QT[e,q] = Wq^T X^T (weights as lhsT),
     V[k,e] = Xk Wv (Xk^T chunks as lhsT). bf16 in/out, fp32 PSUM.
  B: sT[k,q] = KT-tile^T @ QT (scores transposed: own keys on partitions)
     w = exp(sT/32) * band   (band only on the two diagonal q-tiles)
     den[q] = ones-matmul over w; num[q,e] = w-as-lhsT @ V
The transposed-score layout makes the attention weights directly usable as
matmul lhsT for the numerator - no on-chip transposes at all.
"""

import sys

if "/opt/trn_rl_repo" not in sys.path:
    sys.path.insert(0, "/opt/trn_rl_repo")

import numpy as np

B, S, D = 4, 2048, 1024
HK = S // 2  # own key rows per core
P = 128
N_CORES = 8
# column offset of attention-weight block j inside the packed wt tile
WOFF = [0] * 9
for _j in range(8):
    WOFF[_j + 1] = WOFF[_j] + (16 - 2 * _j) * P
WTW = WOFF[8]  # 9216

_cache = {}


def _build_nc():
    from concourse import bacc
    import concourse.mybir as mybir
    import concourse.tile as tile

    fp32 = mybir.dt.float32
    bf16 = mybir.dt.bfloat16
    Exp = mybir.ActivationFunctionType.Exp

    nc = bacc.Bacc("TRN2", target_bir_lowering=False)

    xk_d = nc.dram_tensor("xk", [D, HK], bf16, kind="ExternalInput")
    xq_d = nc.dram_tensor("xq", [D, S], bf16, kind="ExternalInput")
    wq_d = nc.dram_tensor("wq", [D, D], bf16, kind="ExternalInput")
    wk_d = nc.dram_tensor("wk", [D, D], bf16, kind="ExternalInput")
    wv_d = nc.dram_tensor("wv", [D, D], bf16, kind="ExternalInput")
    band_d = nc.dram_tensor("band", [P, 256], bf16, kind="ExternalInput")
    num_d = nc.dram_tensor("num", [S, D], fp32, kind="ExternalOutput")
    den_d = nc.dram_tensor("den", [P, 16], fp32, kind="ExternalOutput")

    xk3 = xk_d.rearrange("(o p) s -> p o s", p=P)
    xq3 = xq_d.rearrange("(o p) q -> p o q", p=P)
    wq3 = wq_d.rearrange("(o p) e -> p o e", p=P)
    wk3 = wk_d.rearrange("(o p) e -> p o e", p=P)
    wv3 = wv_d.rearrange("(o p) e -> p o e", p=P)

    with tile.TileContext(nc) as tc:
        with tc.tile_pool(name="persist", bufs=1) as persist:
            QT = persist.tile([P, 8, S], bf16, tag="qt")
            KT = persist.tile([P, 8, HK], bf16, tag="kt")
            V = persist.tile([P, 8, D], bf16, tag="v")
            band = persist.tile([P, 256], bf16, tag="band")
            ones = persist.tile([P, 1], bf16, tag="ones")
            nc.vector.memset(ones[:], 1.0)

            # ---- Phase A: projections (all bf16 matmuls, fp32 PSUM) ----
            with (
                tc.tile_pool(name="wts", bufs=1) as wp,
                tc.tile_pool(name="xts", bufs=2) as xtsp,
                tc.tile_pool(name="psA", bufs=8, space="PSUM") as psA,
            ):
                Wk = wp.tile([P, 8, D], bf16, tag="wk")
                Wv = wp.tile([P, 8, D], bf16, tag="wv")
                Wq = wp.tile([P, 8, D], bf16, tag="wq")

                for sc in range(2):
                    xs = xtsp.tile([P, 8, 512], bf16, tag="xts")
                    nc.sync.dma_start(xs[:], xk3[:, :, sc * 512 : (sc + 1) * 512])
                    # DMA-issue-order pacing: X chunk first, then the weights
                    # needed soonest; the rest trickle in behind compute.
                    # Batched DMAs: each dma_start costs ~650ns of serial
                    # queue-issue, so fewer+bigger wins.
                    if sc == 0:
                        # Wk halves: the first 4 e-tiles' matmuls need only
                        # the first half, so they start ~3us sooner.
                        nc.sync.dma_start(Wk[:, :, :512], wk3[:, :, :512])
                        nc.sync.dma_start(Wk[:, :, 512:], wk3[:, :, 512:])
                        nc.sync.dma_start(Wv[:], wv3[:])
                    elif sc == 1:
                        nc.sync.dma_start(Wq[:], wq3[:])
                        nc.sync.dma_start(band[:], band_d[:])
                    # KT[e, k-chunk] = Wk^T @ Xk^T chunk
                    for e in range(8):
                        psum = psA.tile([P, 512], fp32, tag="psA")
                        for d in range(8):
                            nc.tensor.matmul(
                                psum[:],
                                Wk[:, d, e * P : (e + 1) * P],
                                xs[:, d],
                                start=(d == 0),
                                stop=(d == 7),
                            )
                        nc.any.tensor_copy(
                            out=KT[:, e, sc * 512 : (sc + 1) * 512], in_=psum[:]
                        )
                    # V[k-tile, e] = Xk chunk @ Wv  (Xk^T slice as lhsT)
                    for kti in range(4):
                        kt = 4 * sc + kti
                        for ec in range(2):
                            psum = psA.tile([P, 512], fp32, tag="psA")
                            for d in range(8):
                                nc.tensor.matmul(
                                    psum[:],
                                    xs[:, d, kti * P : (kti + 1) * P],
                                    Wv[:, d, ec * 512 : (ec + 1) * 512],
                                    start=(d == 0),
                                    stop=(d == 7),
                                )
                            nc.any.tensor_copy(
                                out=V[:, kt, ec * 512 : (ec + 1) * 512], in_=psum[:]
                            )
                # QT[e, q-chunk] = Wq^T @ X^T chunk (all 2048 query rows)
                for qsc in range(4):
                    xs = xtsp.tile([P, 8, 512], bf16, tag="xts")
                    nc.sync.dma_start(xs[:], xq3[:, :, qsc * 512 : (qsc + 1) * 512])
                    for e in range(8):
                        psum = psA.tile([P, 512], fp32, tag="psA")
                        for d in range(8):
                            nc.tensor.matmul(
                                psum[:],
                                Wq[:, d, e * P : (e + 1) * P],
                                xs[:, d],
                                start=(d == 0),
                                stop=(d == 7),
                            )
                        nc.any.tensor_copy(
                            out=QT[:, e, qsc * 512 : (qsc + 1) * 512], in_=psum[:]
                        )

            # ---- Phase B: partial attention over own key tiles ----
            with (
                tc.tile_pool(name="wtp", bufs=1) as wtp,
                tc.tile_pool(name="outp", bufs=2) as outp,
                tc.tile_pool(name="small", bufs=2) as smallp,
                tc.tile_pool(name="psS", bufs=2, space="PSUM") as psS,
                tc.tile_pool(name="psAV", bufs=4, space="PSUM") as psAV,
                tc.tile_pool(name="psD", bufs=2, space="PSUM") as psD,
            ):
                # Interleaved: after key tile j's scores are exp'd, emit the
                # numerator/denominator for global q-tiles g = 2j and 2j+1
                # (they need only key tiles <= j). Keeps PE dense and spreads
                # the PSUM->SBUF copies across the whole phase.
                wt = wtp.tile([P, WTW], bf16, tag="wt")
                den_sb = smallp.tile([P, 16], fp32, tag="den")
                for j in range(8):
                    # scores + exp for own key tile j; q-range [256j, 2048)
                    for ch in range(8 - j):
                        q0 = 256 * j + 256 * ch
                        psum_s = psS.tile([P, 256], fp32, tag="psS")
                        for e in range(8):
                            nc.tensor.matmul(
                                psum_s[:],
                                KT[:, e, j * P : (j + 1) * P],
                                QT[:, e, q0 : q0 + 256],
                                start=(e == 0),
                                stop=(e == 7),
                            )
                        wcol = WOFF[j] + 256 * ch
                        nc.scalar.activation(
                            wt[:, wcol : wcol + 256], psum_s[:], Exp, scale=1 / 32.0
                        )
                        if ch == 0:
                            # diagonal block: causal 0/1 mask (parity in data)
                            nc.vector.tensor_mul(
                                wt[:, wcol : wcol + 256],
                                wt[:, wcol : wcol + 256],
                                band[:],
                            )
                    for g in (2 * j, 2 * j + 1):
                        nj = g // 2 + 1  # own key tiles jj with 2jj <= g
                        out_sb = outp.tile([P, D], fp32, tag="out")
                        psum_dn = psD.tile([P, 1], fp32, tag="psD")
                        for jj in range(nj):
                            nc.tensor.matmul(
                                psum_dn[:],
                                wt[:, WOFF[jj] + (g - 2 * jj) * P :][:, :P],
                                ones[:],
                                start=(jj == 0),
                                stop=(jj == nj - 1),
                            )
                        nc.any.tensor_copy(out=den_sb[:, g : g + 1], in_=psum_dn[:])
                        for ec in range(2):
                            psum_av = psAV.tile([P, 512], fp32, tag="psAV")
                            for jj in range(nj):
                                nc.tensor.matmul(
                                    psum_av[:],
                                    wt[:, WOFF[jj] + (g - 2 * jj) * P :][:, :P],
                                    V[:, jj, ec * 512 : (ec + 1) * 512],
                                    start=(jj == 0),
                                    stop=(jj == nj - 1),
                                )
                            nc.any.tensor_copy(
                                out=out_sb[:, ec * 512 : (ec + 1) * 512],
                                in_=psum_av[:],
                            )
                        nc.sync.dma_start(num_d[g * P : (g + 1) * P, :], out_sb[:])
                nc.sync.dma_start(den_d[:], den_sb[:])

    nc.compile()
    return nc


def _get_nc():
    if "nc" not in _cache:
        _cache["nc"] = _build_nc()
    return _cache["nc"]


def _parity_cols(h):
    return np.concatenate(
        [np.arange(P * (2 * j + h), P * (2 * j + h) + P) for j in range(8)]
    )


def kernel(X, W_q, W_k, W_v, _run_kwargs=None, _results_out=None):
    import ml_dtypes
    from concourse.bass_utils import run_bass_kernel_spmd

    bf = ml_dtypes.bfloat16
    X = np.asarray(X, dtype=np.float32)
    wq16 = np.asarray(W_q, dtype=np.float32).astype(bf)
    wk16 = np.asarray(W_k, dtype=np.float32).astype(bf)
    wv16 = np.asarray(W_v, dtype=np.float32).astype(bf)

    xqs = [np.ascontiguousarray(X[b].T).astype(bf) for b in range(B)]
    cols = [_parity_cols(0), _parity_cols(1)]
    bands = []
    for h in range(2):
        x = np.arange(256)[None, :]
        p = np.arange(P)[:, None]
        bands.append((x >= p + P * h).astype(bf))

    in_maps = []
    for c in range(N_CORES):
        b, h = divmod(c, 2)
        in_maps.append(
            {
                "xk": np.ascontiguousarray(xqs[b][:, cols[h]]),
                "xq": xqs[b],
                "wq": wq16,
                "wk": wk16,
                "wv": wv16,
                "band": bands[h],
            }
        )

    nc = _get_nc()
    res = run_bass_kernel_spmd(
        nc, in_maps, core_ids=list(range(N_CORES)), **(_run_kwargs or {})
    )
    if _results_out is not None:
        _results_out.append(res)

    out = np.empty((B, S, D), dtype=np.float32)
    for b in range(B):
        re, ro = res.results[2 * b], res.results[2 * b + 1]
        num = re["num"] + ro["num"]
        den = (re["den"] + ro["den"]).T.reshape(S)  # row 128g+p <- den[p, g]
        out[b] = num / den[:, None]
    return out


# revision 13
# speedup vs baseline: 1.6717x; 1.0251x over previous
"""Causal single-head attention on 8 Trainium2 NeuronCores (Bass/Tile).

Problem: X [4, 2048, 1024] f32; W_q/W_k/W_v [1024, 1024] f32.
out[b] = softmax(mask((X[b] Wq)(X[b] Wk)^T / 32)) (X[b] Wv)

Sharding: 8 cores = 4 batches x 2 key-parity halves (partial softmax).
Core c = 2b + h owns batch b's key tiles {2j + h : j = 0..7} (128-row
tiles, interleaved so causal work per local tile j is j-independent across
cores). Each core projects K/V only for its own key tiles (K/V computed
once globally; only Q is duplicated - the cheapest possible duplication),
computes unnormalized partial attention over its keys, and returns the
partial numerator [2048, 1024] plus partial softmax denominators. The host
adds each pair's partials and divides. Since exp needs no max-subtraction
here (|scores/32| < ~4), partial softmax combines exactly.

One uniform SPMD program: per-core differences live in data only (which
key columns of X^T arrive in `xk`, and a [128, 256] 0/1 band mask whose
content encodes the core's parity for the diagonal score tiles).

All matmul contractions keep the contracted dim on partitions:
  A: KT[e,k] = Wk^T Xk^T, QT[e,q] = Wq^T X^T (weights as lhsT),
     V[k,e] = Xk Wv (Xk^T chunks as lhsT). bf16 in/out, fp32 PSUM.
  B: sT[k,q] = KT-tile^T @ QT (scores transposed: own keys on partitions)
     w = exp(sT/32) * band   (band only on the two diagonal q-tiles)
     den[q] = ones-matmul over w; num[q,e] = w-as-lhsT @ V
The transposed-score layout makes the attention weights directly usable as
matmul lhsT for the numerator - no on-chip transposes at all.
"""

import sys

if "/opt/trn_rl_repo" not in sys.path:
    sys.path.insert(0, "/opt/trn_rl_repo")

import numpy as np

B, S, D = 4, 2048, 1024
HK = S // 2  # own key rows per core
P = 128
N_CORES = 8
# column offset of attention-weight block j inside the packed wt tile
WOFF = [0] * 9
for _j in range(8):
    WOFF[_j + 1] = WOFF[_j] + (16 - 2 * _j) * P
WTW = WOFF[8]  # 9216

_cache = {}


def _build_nc():
    from concourse import bacc
    import concourse.mybir as mybir
    import concourse.tile as tile

    fp32 = mybir.dt.float32
    bf16 = mybir.dt.bfloat16
    Exp = mybir.ActivationFunctionType.Exp

    nc = bacc.Bacc("TRN2", target_bir_lowering=False)

    xk_d = nc.dram_tensor("xk", [D, HK], bf16, kind="ExternalInput")
    xq_d = nc.dram_tensor("xq", [D, S], bf16, kind="ExternalInput")
    wq_d = nc.dram_tensor("wq", [D, D], bf16, kind="ExternalInput")
    wk_d = nc.dram_tensor("wk", [D, D], bf16, kind="ExternalInput")
    wv_d = nc.dram_tensor("wv", [D, D], bf16, kind="ExternalInput")
    band_d = nc.dram_tensor("band", [P, 256], bf16, kind="ExternalInput")
    num_d = nc.dram_tensor("num", [S, D], fp32, kind="ExternalOutput")
    den_d = nc.dram_tensor("den", [P, 16], fp32, kind="ExternalOutput")

    xk3 = xk_d.rearrange("(o p) s -> p o s", p=P)
    xq3 = xq_d.rearrange("(o p) q -> p o q", p=P)
    wq3 = wq_d.rearrange("(o p) e -> p o e", p=P)
    wk3 = wk_d.rearrange("(o p) e -> p o e", p=P)
    wv3 = wv_d.rearrange("(o p) e -> p o e", p=P)

    with tile.TileContext(nc) as tc:
        with tc.tile_pool(name="persist", bufs=1) as persist:
            QT = persist.tile([P, 8, S], bf16, tag="qt")
            KT = persist.tile([P, 8, HK], bf16, tag="kt")
            V = persist.tile([P, 8, D], bf16, tag="v")
            band = persist.tile([P, 256], bf16, tag="band")
            ones = persist.tile([P, 1], bf16, tag="ones")
            nc.vector.memset(ones[:], 1.0)

            # ---- Phase A: projections (all bf16 matmuls, fp32 PSUM) ----
            with (
                tc.tile_pool(name="wts", bufs=1) as wp,
                tc.tile_pool(name="xts", bufs=2) as xtsp,
                tc.tile_pool(name="psA", bufs=8, space="PSUM") as psA,
            ):
                Wk = wp.tile([P, 8, D], bf16, tag="wk")
                Wv = wp.tile([P, 8, D], bf16, tag="wv")
                Wq = wp.tile([P, 8, D], bf16, tag="wq")

                for sc in range(2):
                    xs = xtsp.tile([P, 8, 512], bf16, tag="xts")
                    if sc == 0:
                        nc.sync.dma_start(xs[:, :4], xk3[:, :4, :512])
                        nc.sync.dma_start(xs[:, 4:], xk3[:, 4:, :512])
                    else:
                        nc.sync.dma_start(
                            xs[:], xk3[:, :, sc * 512 : (sc + 1) * 512]
                        )
                    # DMA-issue-order pacing: X chunk first, then the weights
                    # needed soonest; the rest trickle in behind compute.
                    # Batched DMAs: each dma_start costs ~650ns of serial
                    # queue-issue, so fewer+bigger wins.
                    if sc == 0:
                        # Wk in pieces: the first e-tile's matmuls need
                        # only the first 128 columns, so PE starts early.
                        nc.sync.dma_start(Wk[:, :, :P], wk3[:, :, :P])
                        nc.sync.dma_start(Wk[:, :, P:512], wk3[:, :, P:512])
                        nc.sync.dma_start(Wk[:, :, 512:], wk3[:, :, 512:])
                        nc.sync.dma_start(Wv[:], wv3[:])
                    elif sc == 1:
                        nc.sync.dma_start(Wq[:], wq3[:])
                        nc.sync.dma_start(band[:], band_d[:])
                    # KT[e, k-chunk] = Wk^T @ Xk^T chunk
                    for e in range(8):
                        psum = psA.tile([P, 512], fp32, tag="psA")
                        for d in range(8):
                            nc.tensor.matmul(
                                psum[:],
                                Wk[:, d, e * P : (e + 1) * P],
                                xs[:, d],
                                start=(d == 0),
                                stop=(d == 7),
                            )
                        nc.any.tensor_copy(
                            out=KT[:, e, sc * 512 : (sc + 1) * 512], in_=psum[:]
                        )
                    # V[k-tile, e] = Xk chunk @ Wv  (Xk^T slice as lhsT)
                    for kti in range(4):
                        kt = 4 * sc + kti
                        for ec in range(2):
                            psum = psA.tile([P, 512], fp32, tag="psA")
                            for d in range(8):
                                nc.tensor.matmul(
                                    psum[:],
                                    xs[:, d, kti * P : (kti + 1) * P],
                                    Wv[:, d, ec * 512 : (ec + 1) * 512],
                                    start=(d == 0),
                                    stop=(d == 7),
                                )
                            nc.any.tensor_copy(
                                out=V[:, kt, ec * 512 : (ec + 1) * 512], in_=psum[:]
                            )
                # QT[e, q-chunk] = Wq^T @ X^T chunk (all 2048 query rows)
                for qsc in range(4):
                    xs = xtsp.tile([P, 8, 512], bf16, tag="xts")
                    nc.sync.dma_start(xs[:], xq3[:, :, qsc * 512 : (qsc + 1) * 512])
                    for e in range(8):
                        psum = psA.tile([P, 512], fp32, tag="psA")
                        for d in range(8):
                            nc.tensor.matmul(
                                psum[:],
                                Wq[:, d, e * P : (e + 1) * P],
                                xs[:, d],
                                start=(d == 0),
                                stop=(d == 7),
                            )
                        nc.any.tensor_copy(
                            out=QT[:, e, qsc * 512 : (qsc + 1) * 512], in_=psum[:]
                        )

            # ---- Phase B: partial attention over own key tiles ----
            with (
                tc.tile_pool(name="wtp", bufs=1) as wtp,
                tc.tile_pool(name="outp", bufs=2) as outp,
                tc.tile_pool(name="small", bufs=2) as smallp,
                tc.tile_pool(name="psS", bufs=3, space="PSUM") as psS,
                tc.tile_pool(name="psAV", bufs=4, space="PSUM") as psAV,
                tc.tile_pool(name="psD", bufs=1, space="PSUM") as psD,
            ):
                # Interleaved: after key tile j's scores are exp'd, emit the
                # numerator/denominator for global q-tiles g = 2j and 2j+1
                # (they need only key tiles <= j). Keeps PE dense and spreads
                # the PSUM->SBUF copies across the whole phase.
                wt = wtp.tile([P, WTW], bf16, tag="wt")
                den_sb = smallp.tile([P, 16], fp32, tag="den")
                for j in range(8):
                    # scores + exp for own key tile j; q-range [256j, 2048)
                    for ch in range(8 - j):
                        q0 = 256 * j + 256 * ch
                        psum_s = psS.tile([P, 256], fp32, tag="psS")
                        for e in range(8):
                            nc.tensor.matmul(
                                psum_s[:],
                                KT[:, e, j * P : (j + 1) * P],
                                QT[:, e, q0 : q0 + 256],
                                start=(e == 0),
                                stop=(e == 7),
                            )
                        wcol = WOFF[j] + 256 * ch
                        nc.scalar.activation(
                            wt[:, wcol : wcol + 256], psum_s[:], Exp, scale=1 / 32.0
                        )
                        if ch == 0:
                            # diagonal block: causal 0/1 mask (parity in data)
                            nc.vector.tensor_mul(
                                wt[:, wcol : wcol + 256],
                                wt[:, wcol : wcol + 256],
                                band[:],
                            )
                    for g in (2 * j, 2 * j + 1):
                        nj = g // 2 + 1  # own key tiles jj with 2jj <= g
                        out_sb = outp.tile([P, D], fp32, tag="out")
                        psum_dn = psD.tile([P, 1], fp32, tag="psD")
                        for jj in range(nj):
                            nc.tensor.matmul(
                                psum_dn[:],
                                wt[:, WOFF[jj] + (g - 2 * jj) * P :][:, :P],
                                ones[:],
                                start=(jj == 0),
                                stop=(jj == nj - 1),
                            )
                        nc.any.tensor_copy(out=den_sb[:, g : g + 1], in_=psum_dn[:])
                        for ec in range(2):
                            psum_av = psAV.tile([P, 512], fp32, tag="psAV")
                            for jj in range(nj):
                                nc.tensor.matmul(
                                    psum_av[:],
                                    wt[:, WOFF[jj] + (g - 2 * jj) * P :][:, :P],
                                    V[:, jj, ec * 512 : (ec + 1) * 512],
                                    start=(jj == 0),
                                    stop=(jj == nj - 1),
                                )
                            nc.any.tensor_copy(
                                out=out_sb[:, ec * 512 : (ec + 1) * 512],
                                in_=psum_av[:],
                            )
                        nc.sync.dma_start(num_d[g * P : (g + 1) * P, :], out_sb[:])
                nc.sync.dma_start(den_d[:], den_sb[:])

    nc.compile()
    return nc


def _get_nc():
    if "nc" not in _cache:
        _cache["nc"] = _build_nc()
    return _cache["nc"]


def _parity_cols(h):
    return np.concatenate(
        [np.arange(P * (2 * j + h), P * (2 * j + h) + P) for j in range(8)]
    )


def kernel(X, W_q, W_k, W_v, _run_kwargs=None, _results_out=None):
    import ml_dtypes
    from concourse.bass_utils import run_bass_kernel_spmd

    bf = ml_dtypes.bfloat16
    X = np.asarray(X, dtype=np.float32)
    wq16 = np.asarray(W_q, dtype=np.float32).astype(bf)
    wk16 = np.asarray(W_k, dtype=np.float32).astype(bf)
    wv16 = np.asarray(W_v, dtype=np.float32).astype(bf)

    xqs = [np.ascontiguousarray(X[b].T).astype(bf) for b in range(B)]
    cols = [_parity_cols(0), _parity_cols(1)]
    bands = []
    for h in range(2):
        x = np.arange(256)[None, :]
        p = np.arange(P)[:, None]
        bands.append((x >= p + P * h).astype(bf))

    in_maps = []
    for c in range(N_CORES):
        b, h = divmod(c, 2)
        in_maps.append(
            {
                "xk": np.ascontiguousarray(xqs[b][:, cols[h]]),
                "xq": xqs[b],
                "wq": wq16,
                "wk": wk16,
                "wv": wv16,
                "band": bands[h],
            }
        )

    nc = _get_nc()
    res = run_bass_kernel_spmd(
        nc, in_maps, core_ids=list(range(N_CORES)), **(_run_kwargs or {})
    )
    if _results_out is not None:
        _results_out.append(res)

    out = np.empty((B, S, D), dtype=np.float32)
    for b in range(B):
        re, ro = res.results[2 * b], res.results[2 * b + 1]
        num = re["num"] + ro["num"]
        den = (re["den"] + ro["den"]).T.reshape(S)  # row 128g+p <- den[p, g]
        out[b] = num / den[:, None]
    return out
